# revision 39
# baseline (speedup 1.0000x reference)
"""Trainium2 Bass kernel for HCEN forward: out = ((x.mean(axis=1)) @ W_enc.T + b_enc) @ W_out.T + b_out.

Sharding: data-parallel over batch. B=16 across 8 cores -> 2 batches/core.
Weights replicated per core. No collectives.

v3: the seq-mean is computed by FOUR engines in parallel, with x shipped in
two host-prepared forms per batch:
  - seq rows [0, S_PE): fp8(e4m3) in [B, S_PE, D] layout. The PE reduces them
    with ones-stationary matmuls (moving [128, 512] fp8, ~0.21us ramped) into
    PSUM rows pe_ps[b, :]. This also keeps the PE out of low pstate for the
    layer-1/2 tail. The per-batch [1, 1024] partials are PE-transposed into
    tp_all[128, (c,b)] with a [1,1]-"identity" holding 1/S, folding the mean
    scale in for free.
  - seq rows [S_PE, S): int8 (scale qs=|x|.max()/127) in [B, D, S-S_PE]
    layout, d on partitions. Per 128-d chunk one free-axis reduction:
    ACT activation(Copy, accum_out) / DVE tensor_reduce / gpsimd+DVE team
    (gps folds halves i8+i8->i16, DVE reduces the i16 half).
Raw sums land in parts[128, 16] f32 (exact) and tp_all (PSUM); the mean is
mt_bf = parts*(qs/S) + tp_all in two DVE ops (qs ships as a tiny input so the
compiled program is input-independent). Layer 1/2 run c-outer/n-inner so each
weight chunk (queued after x on the same HWDGE ring) is consumed on arrival.
"""

import os
import sys
from contextlib import ExitStack

import ml_dtypes
import numpy as np

for _p in ("/opt/trn_rl_repo", "/root/.axon_site/_ro/trn_rl_repo"):
    if os.path.isdir(_p) and _p not in sys.path:
        sys.path.insert(0, _p)

import concourse.bass as bass  # noqa: E402
import concourse.tile as tile  # noqa: E402
from concourse import bacc, mybir  # noqa: E402
from concourse.bass_utils import run_bass_kernel_spmd  # noqa: E402
from concourse.masks import make_identity  # noqa: E402

B, S, D, H, O = 16, 4096, 1024, 1024, 1024
NCORES = 8
BPC = B // NCORES  # batches per core
P = 128
DC = D // P  # 8 d-chunks
HC = H // P
NF = 512  # matmul moving free dim (PSUM bank limit)

QPE = 13  # 128-row seq subtiles handled by the PE (per batch)
S_PE = QPE * P  # 1664
S_R = S - S_PE  # 2432 seq rows to ACT/DVE/gps, int8
UT = 4  # d-chunks per int8 DMA tile -> [128, UT, S_R] = 1.25 MB
NTB = DC // UT  # 4 int8 tiles per batch

F32 = mybir.dt.float32
BF16 = mybir.dt.bfloat16
FP8 = mybir.dt.float8e4
I8 = mybir.dt.int8
I16 = mybir.dt.int16

# per-batch engine assignment for the 8 int8 d-chunks:
# 'A' -> ACT (~2.2us), 'D' -> DVE tensor_reduce (~2.7us),
# 'T' -> team: gps fold1 (~2.4us) + DVE reduce of the i16 half (~1.3us)
_ASSIGN = [
    ["A", "T", "D", "A", "T", "T", "A", "T"],  # batch 0
    ["A", "T", "D", "T", "A", "D", "T", "A"],  # batch 1
]


def build_nc():
    nc = bacc.Bacc(
        "TRN2",
        target_bir_lowering=False,
        debug=False,
        enable_asserts=False,
        num_devices=NCORES,
    )
    xpe_ext = nc.dram_tensor("xpe", [BPC, S_PE, D], FP8, kind="ExternalInput").ap()
    x8_ext = nc.dram_tensor("x8", [BPC, D, S_R], I8, kind="ExternalInput").ap()
    qs_ext = nc.dram_tensor("qs", [1], F32, kind="ExternalInput").ap()
    wencT_ext = nc.dram_tensor("wencT", [D, H], BF16, kind="ExternalInput").ap()
    woutT_ext = nc.dram_tensor("woutT", [H, O], BF16, kind="ExternalInput").ap()
    benc_ext = nc.dram_tensor("benc", [H], F32, kind="ExternalInput").ap()
    bout_ext = nc.dram_tensor("bout", [O], F32, kind="ExternalInput").ap()
    out_ext = nc.dram_tensor("out", [BPC, O], F32, kind="ExternalOutput").ap()

    with ExitStack() as ctx:
        tc = ctx.enter_context(tile.TileContext(nc))
        consts = ctx.enter_context(tc.tile_pool(name="consts", bufs=1))
        wpool = ctx.enter_context(tc.tile_pool(name="wpool", bufs=1))
        xpool = ctx.enter_context(tc.tile_pool(name="xpool", bufs=8))
        pepool = ctx.enter_context(tc.tile_pool(name="pepool", bufs=2))
        gpool = ctx.enter_context(tc.tile_pool(name="gpool", bufs=2))
        spool = ctx.enter_context(tc.tile_pool(name="spool", bufs=1))
        pp2 = ctx.enter_context(tc.tile_pool(name="pp2", bufs=1, space="PSUM"))
        ppe = ctx.enter_context(tc.tile_pool(name="ppe", bufs=2, space="PSUM"))

        ident2 = consts.tile([BPC, BPC], F32)
        make_identity(nc, ident2[:])
        ones8 = consts.tile([P, 1], FP8)
        nc.gpsimd.memset(ones8[:], 1.0)
        ident1 = consts.tile([1, 1], F32)  # true identity for [1,128] transposes
        nc.gpsimd.memset(ident1[:], 1.0)

        g_act = spool.tile([P, S_R], I8, name="g_act")  # ACT copy sink
        parts = spool.tile([P, DC * BPC], F32, name="parts")
        nc.gpsimd.memset(parts[:], 0.0)
        # per-batch [1, D] partials at base partition 0 (matmul/transpose
        # operands must start at partition 0/32/64)
        pe_sbs = [spool.tile([1, D], F32, name=f"pe_sb{b}") for b in range(BPC)]

        tp_all = pp2.tile([P, DC * BPC], F32, name="tp_all", tag="tpall")

        for b in range(BPC):
            # fp8 part: PE ones-matmul reduction over S_PE seq rows
            xpe = pepool.tile([P, QPE, D], FP8, name="xpe", tag="xpe")
            nc.sync.dma_start(
                xpe[:],
                xpe_ext[b, :, :].rearrange("(p q) d -> p q d", p=P),
            )
            pe_ps = ppe.tile([1, D], F32, name=f"pe_ps{b}", tag="peps")
            # q-outer / n-inner so consecutive MMs alternate PSUM banks
            # (same-bank accumulation serializes on writeback)
            for q in range(QPE):
                for n in range(D // NF):
                    nc.tensor.matmul(
                        pe_ps[:, n * NF : (n + 1) * NF],
                        ones8[:],
                        xpe[:, q, n * NF : (n + 1) * NF],
                        start=(q == 0),
                        stop=(q == QPE - 1),
                    )
            # int8 part: per-chunk free-axis reductions on ACT/DVE/gps
            for t in range(NTB):
                xt = xpool.tile([P, UT, S_R], I8, name="xt", tag="xt")
                nc.sync.dma_start(
                    xt[:],
                    x8_ext[b, t * UT * P : (t + 1) * UT * P, :].rearrange(
                        "(u p) s -> p u s", p=P
                    ),
                )
                for u in range(UT):
                    c = t * UT + u
                    col = c * BPC + b
                    kind = _ASSIGN[b][c]
                    if kind == "A":
                        nc.scalar.activation(
                            g_act[:],
                            xt[:, u, :],
                            mybir.ActivationFunctionType.Copy,
                            accum_out=parts[:, col : col + 1],
                        )
                    elif kind == "D":
                        nc.vector.tensor_reduce(
                            parts[:, col : col + 1],
                            xt[:, u, :],
                            op=mybir.AluOpType.add,
                            axis=mybir.AxisListType.X,
                        )
                    else:  # team: gps folds halves i8+i8->bf16 (exact to
                        # +-254; Pool int ops require matching dtypes), DVE
                        # reduces the folded half
                        g16 = gpool.tile([P, S_R // 2], BF16, name="g16", tag="g16")
                        nc.gpsimd.tensor_add(
                            g16[:], xt[:, u, 0 : S_R // 2], xt[:, u, S_R // 2 : S_R]
                        )
                        nc.vector.tensor_reduce(
                            parts[:, col : col + 1],
                            g16[:],
                            op=mybir.AluOpType.add,
                            axis=mybir.AxisListType.X,
                        )
            # move this batch's PE partial out of PSUM (PE and gps can't read
            # PSUM), folding in the 1/S mean scale; ACT has the most slack
            nc.scalar.mul(pe_sbs[b][:], pe_ps[:], 1.0 / S)

        # PE-transpose the [1, 1024] partials into tp_all[128, (c,b)], scaling
        # by 1/S via the identity value.
        for b in range(BPC):
            for c in range(DC):
                nc.tensor.transpose(
                    tp_all[:, c * BPC + b : c * BPC + b + 1],
                    pe_sbs[b][:, c * P : (c + 1) * P],
                    ident1[:],
                )

        # ---- small consts + weights (queued after x) ----
        qs_bc = consts.tile([P, 1], F32, name="qs_bc")
        nc.sync.dma_start(qs_bc[:], qs_ext[None, :].broadcast_to([P, 1]))
        benc2 = consts.tile([BPC, H], F32, name="benc2")
        nc.sync.dma_start(benc2[:], benc_ext[None, :].broadcast_to([BPC, H]))
        bout2 = consts.tile([BPC, O], F32, name="bout2")
        nc.sync.dma_start(bout2[:], bout_ext[None, :].broadcast_to([BPC, O]))
        wenc_sb = wpool.tile([P, DC, H], BF16)
        nc.sync.dma_start(
            wenc_sb[:], wencT_ext[:, :].rearrange("(c p) h -> p c h", p=P)
        )
        wout_sb = wpool.tile([P, HC, O], BF16)
        nc.sync.dma_start(
            wout_sb[:], woutT_ext[:, :].rearrange("(c p) h -> p c h", p=P)
        )

        # ---- mT = parts*(qs/S) + tp_all  -> bf16 [128, (c,b)] ----
        tmp_f = spool.tile([P, DC * BPC], F32, name="tmp_f")
        nc.vector.tensor_scalar_mul(tmp_f[:], parts[:], qs_bc[:])
        mt_bf = spool.tile([P, DC * BPC], BF16, name="mt_bf")
        nc.vector.tensor_add(mt_bf[:], tmp_f[:], tp_all[:])

        # ---- layer 1 ----
        enc_ps = pp2.tile([BPC, H], F32, name="enc_ps", tag="ps2")
        enc_sb = spool.tile([BPC, H], F32, name="enc_sb")
        for c in range(DC):
            for n in range(H // NF):
                nc.tensor.matmul(
                    enc_ps[:, n * NF : (n + 1) * NF],
                    mt_bf[:, c * BPC : (c + 1) * BPC],
                    wenc_sb[:, c, n * NF : (n + 1) * NF],
                    start=(c == 0),
                    stop=(c == DC - 1),
                )
        for n in range(H // NF):
            sl = slice(n * NF, (n + 1) * NF)
            nc.vector.tensor_add(enc_sb[:, sl], enc_ps[:, sl], benc2[:, sl])

        # ---- transpose enc -> encT via PE, all 8 chunks into one PSUM
        # tile's columns, then a single DVE copy to bf16 ----
        encT_sb = spool.tile([P, HC, BPC], BF16, name="encT_sb")
        tpE = pp2.tile([P, HC * BPC], F32, name="tpE", tag="tpE")
        for c in range(HC):
            nc.tensor.transpose(
                tpE[:, c * BPC : (c + 1) * BPC],
                enc_sb[:, c * P : (c + 1) * P],
                ident2[:],
            )
        nc.vector.tensor_copy(encT_sb[:].rearrange("p c b -> p (c b)"), tpE[:])

        # ---- layer 2 ----
        out_ps = pp2.tile([BPC, O], F32, name="out_ps", tag="ps2")
        out_sb = spool.tile([BPC, O], F32, name="out_sb")
        for c in range(HC):
            for n in range(O // NF):
                nc.tensor.matmul(
                    out_ps[:, n * NF : (n + 1) * NF],
                    encT_sb[:, c, :],
                    wout_sb[:, c, n * NF : (n + 1) * NF],
                    start=(c == 0),
                    stop=(c == HC - 1),
                )
        for n in range(O // NF):
            sl = slice(n * NF, (n + 1) * NF)
            nc.vector.tensor_add(out_sb[:, sl], out_ps[:, sl], bout2[:, sl])
        nc.sync.dma_start(out_ext[:], out_sb[:])

    nc.compile()
    return nc


_CACHE = {}


def _cached_nc():
    if "nc" not in _CACHE:
        _CACHE["nc"] = build_nc()
    return _CACHE["nc"]


def make_in_maps(x, W_enc, b_enc, W_out, b_out):
    x = np.asarray(x, dtype=np.float32)
    qs = float(np.abs(x).max()) / 127.0
    xpe = np.ascontiguousarray(x[:, :S_PE, :].astype(ml_dtypes.float8_e4m3fn))
    x8 = np.ascontiguousarray(
        np.rint(x[:, S_PE:, :] * (1.0 / qs)).astype(np.int8).transpose(0, 2, 1)
    )  # [B, D, S_R]
    qs_arr = np.array([qs / S], dtype=np.float32)
    wencT = np.ascontiguousarray(
        np.asarray(W_enc, dtype=np.float32).T.astype(ml_dtypes.bfloat16)
    )
    woutT = np.ascontiguousarray(
        np.asarray(W_out, dtype=np.float32).T.astype(ml_dtypes.bfloat16)
    )
    benc = np.ascontiguousarray(np.asarray(b_enc, dtype=np.float32))
    bout = np.ascontiguousarray(np.asarray(b_out, dtype=np.float32))
    return [
        {
            "xpe": xpe[i * BPC : (i + 1) * BPC],
            "x8": x8[i * BPC : (i + 1) * BPC],
            "qs": qs_arr,
            "wencT": wencT,
            "woutT": woutT,
            "benc": benc,
            "bout": bout,
        }
        for i in range(NCORES)
    ]


def gather_out(results):
    return np.ascontiguousarray(
        np.concatenate([results[i]["out"] for i in range(NCORES)], axis=0)
    )


def kernel(x, W_enc, b_enc, W_out, b_out):
    nc = _cached_nc()
    in_maps = make_in_maps(x, W_enc, b_enc, W_out, b_out)
    res = run_bass_kernel_spmd(nc, in_maps, list(range(NCORES)))
    return gather_out(res.results)


# revision 40
# speedup vs baseline: 1.0670x; 1.0670x over previous
"""Trainium2 Bass kernel for HCEN forward: out = ((x.mean(axis=1)) @ W_enc.T + b_enc) @ W_out.T + b_out.

Sharding: data-parallel over batch. B=16 across 8 cores -> 2 batches/core.
Weights replicated per core. No collectives.

v3: the seq-mean is computed by FOUR engines in parallel, with x shipped in
two host-prepared forms per batch:
  - seq rows [0, S_PE): fp8(e4m3) in [B, S_PE, D] layout. The PE reduces them
    with ones-stationary matmuls (moving [128, 512] fp8, ~0.21us ramped) into
    PSUM rows pe_ps[b, :]. This also keeps the PE out of low pstate for the
    layer-1/2 tail. The per-batch [1, 1024] partials are PE-transposed into
    tp_all[128, (c,b)] with a [1,1]-"identity" holding 1/S, folding the mean
    scale in for free.
  - seq rows [S_PE, S): int8 (scale qs=|x|.max()/127) in [B, D, S-S_PE]
    layout, d on partitions. Per 128-d chunk one free-axis reduction:
    ACT activation(Copy, accum_out) / DVE tensor_reduce / gpsimd+DVE team
    (gps folds halves i8+i8->i16, DVE reduces the i16 half).
Raw sums land in parts[128, 16] f32 (exact) and tp_all (PSUM); the mean is
mt_bf = parts*(qs/S) + tp_all in two DVE ops (qs ships as a tiny input so the
compiled program is input-independent). Layer 1/2 run c-outer/n-inner so each
weight chunk (queued after x on the same HWDGE ring) is consumed on arrival.
"""

import os
import sys
from contextlib import ExitStack

import ml_dtypes
import numpy as np

for _p in ("/opt/trn_rl_repo", "/root/.axon_site/_ro/trn_rl_repo"):
    if os.path.isdir(_p) and _p not in sys.path:
        sys.path.insert(0, _p)

import concourse.bass as bass  # noqa: E402
import concourse.tile as tile  # noqa: E402
from concourse import bacc, mybir  # noqa: E402
from concourse.bass_utils import run_bass_kernel_spmd  # noqa: E402
from concourse.masks import make_identity  # noqa: E402

B, S, D, H, O = 16, 4096, 1024, 1024, 1024
NCORES = 8
BPC = B // NCORES  # batches per core
P = 128
DC = D // P  # 8 d-chunks
HC = H // P
NF = 512  # matmul moving free dim (PSUM bank limit)

QPE = 13  # 128-row seq subtiles handled by the PE (per batch)
S_PE = QPE * P  # 1664
S_R = S - S_PE  # 2432 seq rows to ACT/DVE/gps, int8
UT = 2  # d-chunks per int8 DMA tile -> [128, UT, S_R] = 623 KB
NTB = DC // UT  # 4 int8 tiles per batch

F32 = mybir.dt.float32
BF16 = mybir.dt.bfloat16
FP8 = mybir.dt.float8e4
I8 = mybir.dt.int8
I16 = mybir.dt.int16

# per-batch engine assignment for the 8 int8 d-chunks:
# 'A' -> ACT (~2.2us), 'D' -> DVE tensor_reduce (~2.7us),
# 'T' -> team: gps fold1 (~2.4us) + DVE reduce of the i16 half (~1.3us)
_ASSIGN = [
    ["A", "T", "D", "A", "T", "T", "A", "T"],  # batch 0
    ["A", "T", "D", "T", "A", "D", "T", "A"],  # batch 1
]


def build_nc():
    nc = bacc.Bacc(
        "TRN2",
        target_bir_lowering=False,
        debug=False,
        enable_asserts=False,
        num_devices=NCORES,
    )
    xpe_ext = nc.dram_tensor("xpe", [BPC, S_PE, D], FP8, kind="ExternalInput").ap()
    x8_ext = nc.dram_tensor("x8", [BPC, D, S_R], I8, kind="ExternalInput").ap()
    qs_ext = nc.dram_tensor("qs", [1], F32, kind="ExternalInput").ap()
    wencT_ext = nc.dram_tensor("wencT", [D, H], BF16, kind="ExternalInput").ap()
    woutT_ext = nc.dram_tensor("woutT", [H, O], BF16, kind="ExternalInput").ap()
    benc_ext = nc.dram_tensor("benc", [H], F32, kind="ExternalInput").ap()
    bout_ext = nc.dram_tensor("bout", [O], F32, kind="ExternalInput").ap()
    out_ext = nc.dram_tensor("out", [BPC, O], F32, kind="ExternalOutput").ap()

    with ExitStack() as ctx:
        tc = ctx.enter_context(tile.TileContext(nc))
        consts = ctx.enter_context(tc.tile_pool(name="consts", bufs=1))
        wpool = ctx.enter_context(tc.tile_pool(name="wpool", bufs=1))
        xpool = ctx.enter_context(tc.tile_pool(name="xpool", bufs=8))
        pepool = ctx.enter_context(tc.tile_pool(name="pepool", bufs=2))
        gpool = ctx.enter_context(tc.tile_pool(name="gpool", bufs=2))
        spool = ctx.enter_context(tc.tile_pool(name="spool", bufs=1))
        pp2 = ctx.enter_context(tc.tile_pool(name="pp2", bufs=1, space="PSUM"))
        ppe = ctx.enter_context(tc.tile_pool(name="ppe", bufs=2, space="PSUM"))

        ident2 = consts.tile([BPC, BPC], F32)
        make_identity(nc, ident2[:])
        ones8 = consts.tile([P, 1], FP8)
        nc.gpsimd.memset(ones8[:], 1.0)
        ident1 = consts.tile([1, 1], F32)  # true identity for [1,128] transposes
        nc.gpsimd.memset(ident1[:], 1.0)

        g_act = spool.tile([P, S_R], I8, name="g_act")  # ACT copy sink
        parts = spool.tile([P, DC * BPC], F32, name="parts")
        nc.gpsimd.memset(parts[:], 0.0)
        # per-batch [1, D] partials at base partition 0 (matmul/transpose
        # operands must start at partition 0/32/64)
        pe_sbs = [spool.tile([1, D], F32, name=f"pe_sb{b}") for b in range(BPC)]

        tp_all = pp2.tile([P, DC * BPC], F32, name="tp_all", tag="tpall")

        for b in range(BPC):
            # fp8 part: PE ones-matmul reduction over S_PE seq rows
            xpe = pepool.tile([P, QPE, D], FP8, name="xpe", tag="xpe")
            nc.sync.dma_start(
                xpe[:],
                xpe_ext[b, :, :].rearrange("(p q) d -> p q d", p=P),
            )
            pe_ps = ppe.tile([1, D], F32, name=f"pe_ps{b}", tag="peps")
            # q-outer / n-inner so consecutive MMs alternate PSUM banks
            # (same-bank accumulation serializes on writeback)
            for q in range(QPE):
                for n in range(D // NF):
                    nc.tensor.matmul(
                        pe_ps[:, n * NF : (n + 1) * NF],
                        ones8[:],
                        xpe[:, q, n * NF : (n + 1) * NF],
                        start=(q == 0),
                        stop=(q == QPE - 1),
                    )
            # int8 part: per-chunk free-axis reductions on ACT/DVE/gps
            for t in range(NTB):
                xt = xpool.tile([P, UT, S_R], I8, name="xt", tag="xt")
                nc.sync.dma_start(
                    xt[:],
                    x8_ext[b, t * UT * P : (t + 1) * UT * P, :].rearrange(
                        "(u p) s -> p u s", p=P
                    ),
                )
                for u in range(UT):
                    c = t * UT + u
                    col = c * BPC + b
                    kind = _ASSIGN[b][c]
                    if kind == "A":
                        nc.scalar.activation(
                            g_act[:],
                            xt[:, u, :],
                            mybir.ActivationFunctionType.Copy,
                            accum_out=parts[:, col : col + 1],
                        )
                    elif kind == "D":
                        nc.vector.tensor_reduce(
                            parts[:, col : col + 1],
                            xt[:, u, :],
                            op=mybir.AluOpType.add,
                            axis=mybir.AxisListType.X,
                        )
                    else:  # team: gps folds halves i8+i8->bf16 (exact to
                        # +-254; Pool int ops require matching dtypes), DVE
                        # reduces the folded half
                        g16 = gpool.tile([P, S_R // 2], BF16, name="g16", tag="g16")
                        nc.gpsimd.tensor_add(
                            g16[:], xt[:, u, 0 : S_R // 2], xt[:, u, S_R // 2 : S_R]
                        )
                        nc.vector.tensor_reduce(
                            parts[:, col : col + 1],
                            g16[:],
                            op=mybir.AluOpType.add,
                            axis=mybir.AxisListType.X,
                        )
            # move this batch's PE partial out of PSUM (PE and gps can't read
            # PSUM), folding in the 1/S mean scale; ACT has the most slack
            nc.scalar.mul(pe_sbs[b][:], pe_ps[:], 1.0 / S)

        # PE-transpose the [1, 1024] partials into tp_all[128, (c,b)], scaling
        # by 1/S via the identity value.
        for b in range(BPC):
            for c in range(DC):
                nc.tensor.transpose(
                    tp_all[:, c * BPC + b : c * BPC + b + 1],
                    pe_sbs[b][:, c * P : (c + 1) * P],
                    ident1[:],
                )

        # ---- small consts + weights (queued after x) ----
        qs_bc = consts.tile([P, 1], F32, name="qs_bc")
        nc.sync.dma_start(qs_bc[:], qs_ext[None, :].broadcast_to([P, 1]))
        benc2 = consts.tile([BPC, H], F32, name="benc2")
        nc.sync.dma_start(benc2[:], benc_ext[None, :].broadcast_to([BPC, H]))
        bout2 = consts.tile([BPC, O], F32, name="bout2")
        nc.sync.dma_start(bout2[:], bout_ext[None, :].broadcast_to([BPC, O]))
        wenc_sb = wpool.tile([P, DC, H], BF16)
        nc.sync.dma_start(
            wenc_sb[:], wencT_ext[:, :].rearrange("(c p) h -> p c h", p=P)
        )
        wout_sb = wpool.tile([P, HC, O], BF16)
        nc.sync.dma_start(
            wout_sb[:], woutT_ext[:, :].rearrange("(c p) h -> p c h", p=P)
        )

        # ---- mT = parts*(qs/S) + tp_all  -> bf16 [128, (c,b)] ----
        tmp_f = spool.tile([P, DC * BPC], F32, name="tmp_f")
        nc.vector.tensor_scalar_mul(tmp_f[:], parts[:], qs_bc[:])
        mt_bf = spool.tile([P, DC * BPC], BF16, name="mt_bf")
        nc.vector.tensor_add(mt_bf[:], tmp_f[:], tp_all[:])

        # ---- layer 1 ----
        enc_ps = pp2.tile([BPC, H], F32, name="enc_ps", tag="ps2")
        enc_sb = spool.tile([BPC, H], F32, name="enc_sb")
        for c in range(DC):
            for n in range(H // NF):
                nc.tensor.matmul(
                    enc_ps[:, n * NF : (n + 1) * NF],
                    mt_bf[:, c * BPC : (c + 1) * BPC],
                    wenc_sb[:, c, n * NF : (n + 1) * NF],
                    start=(c == 0),
                    stop=(c == DC - 1),
                )
        for n in range(H // NF):
            sl = slice(n * NF, (n + 1) * NF)
            nc.vector.tensor_add(enc_sb[:, sl], enc_ps[:, sl], benc2[:, sl])

        # ---- transpose enc -> encT via PE, all 8 chunks into one PSUM
        # tile's columns, then a single DVE copy to bf16 ----
        encT_sb = spool.tile([P, HC, BPC], BF16, name="encT_sb")
        tpE = pp2.tile([P, HC * BPC], F32, name="tpE", tag="tpE")
        for c in range(HC):
            nc.tensor.transpose(
                tpE[:, c * BPC : (c + 1) * BPC],
                enc_sb[:, c * P : (c + 1) * P],
                ident2[:],
            )
        nc.vector.tensor_copy(encT_sb[:].rearrange("p c b -> p (c b)"), tpE[:])

        # ---- layer 2 ----
        out_ps = pp2.tile([BPC, O], F32, name="out_ps", tag="ps2")
        out_sb = spool.tile([BPC, O], F32, name="out_sb")
        for c in range(HC):
            for n in range(O // NF):
                nc.tensor.matmul(
                    out_ps[:, n * NF : (n + 1) * NF],
                    encT_sb[:, c, :],
                    wout_sb[:, c, n * NF : (n + 1) * NF],
                    start=(c == 0),
                    stop=(c == HC - 1),
                )
        for n in range(O // NF):
            sl = slice(n * NF, (n + 1) * NF)
            nc.vector.tensor_add(out_sb[:, sl], out_ps[:, sl], bout2[:, sl])
        nc.sync.dma_start(out_ext[:], out_sb[:])

    nc.compile()
    return nc


_CACHE = {}


def _cached_nc():
    if "nc" not in _CACHE:
        _CACHE["nc"] = build_nc()
    return _CACHE["nc"]


def make_in_maps(x, W_enc, b_enc, W_out, b_out):
    x = np.asarray(x, dtype=np.float32)
    qs = float(np.abs(x).max()) / 127.0
    xpe = np.ascontiguousarray(x[:, :S_PE, :].astype(ml_dtypes.float8_e4m3fn))
    x8 = np.ascontiguousarray(
        np.rint(x[:, S_PE:, :] * (1.0 / qs)).astype(np.int8).transpose(0, 2, 1)
    )  # [B, D, S_R]
    qs_arr = np.array([qs / S], dtype=np.float32)
    wencT = np.ascontiguousarray(
        np.asarray(W_enc, dtype=np.float32).T.astype(ml_dtypes.bfloat16)
    )
    woutT = np.ascontiguousarray(
        np.asarray(W_out, dtype=np.float32).T.astype(ml_dtypes.bfloat16)
    )
    benc = np.ascontiguousarray(np.asarray(b_enc, dtype=np.float32))
    bout = np.ascontiguousarray(np.asarray(b_out, dtype=np.float32))
    return [
        {
            "xpe": xpe[i * BPC : (i + 1) * BPC],
            "x8": x8[i * BPC : (i + 1) * BPC],
            "qs": qs_arr,
            "wencT": wencT,
            "woutT": woutT,
            "benc": benc,
            "bout": bout,
        }
        for i in range(NCORES)
    ]


def gather_out(results):
    return np.ascontiguousarray(
        np.concatenate([results[i]["out"] for i in range(NCORES)], axis=0)
    )


def kernel(x, W_enc, b_enc, W_out, b_out):
    nc = _cached_nc()
    in_maps = make_in_maps(x, W_enc, b_enc, W_out, b_out)
    res = run_bass_kernel_spmd(nc, in_maps, list(range(NCORES)))
    return gather_out(res.results)
